# revision 5
# baseline (speedup 1.0000x reference)
"""Trainium2 Bass kernel for nn_DialogActLabeller (segment_reduce).

Computes, for input enc_output [32, 4096, 1024], W [1024, 256], b [256],
cls_pos [32, 64], last_sep [32]:

    x = enc_output @ W + b                      # [B, S, 256]
    seg[b, n] = sum_{s in [start_n, end_n)} x[b, s, :]
    out = log_softmax(seg, axis=-1)             # [B, 64, 256]

Key algebraic restructure: the projection is linear, so segment-reduce
FIRST on enc_output (via a matmul with a 0/1 segment-indicator matrix A),
then project the tiny [64, 1024] per-batch result with W, and add
len_n * b for the bias.  This reads enc_output exactly once from HBM and
does ~1/32 of the naive FLOPs.

Sharding: pure data parallel, 4 batch rows per core across 8 cores
(W, b replicated), no cross-core communication.
"""

import os
import numpy as np

import concourse.bacc as bacc
import concourse.bass as bass
import concourse.tile as tile
from concourse import mybir
from concourse import bass_utils
from contextlib import ExitStack

# Problem shapes (hardcoded per contract)
B, S, D_IN, D_OUT, N_SENT = 32, 4096, 1024, 256, 64
N_CORES = 8
BPC = B // N_CORES          # batches per core
SCHUNKS = S // 128          # 32 sequence chunks of 128
DCH = D_IN // 128           # 8 d_in chunks of 128
SS_PER_DMA = 4              # s-chunks per enc DMA (2 MiB transfers)

F32 = mybir.dt.float32

# Matmul dtype for the big segment-reduce matmul and the small projection
# matmul.  float32r streams 4x faster through the PE than float32 on TRN2.
# Both operands are bitcast views of the same fp32 bits.
_SEG_MM_DT = getattr(mybir.dt, os.environ.get("SEG_MM_DT", "float32r"))
_PROJ_MM_DT = getattr(mybir.dt, os.environ.get("PROJ_MM_DT", "float32"))


def _build_program():
    nc = bacc.Bacc("TRN2", debug=False)

    # The segment-reduce matmul operands are declared end-to-end in the
    # matmul dtype (float32r is bit-identical to float32 in memory, so the
    # host still feeds plain fp32 arrays and the DMA is a plain copy).
    enc = nc.dram_tensor("enc", [BPC, S, D_IN], _SEG_MM_DT, kind="ExternalInput").ap()
    wt = nc.dram_tensor("w", [D_IN, D_OUT], F32, kind="ExternalInput").ap()
    bias = nc.dram_tensor("bias", [D_OUT], F32, kind="ExternalInput").ap()
    amat = nc.dram_tensor(
        "amat", [BPC, 128, SCHUNKS * N_SENT], _SEG_MM_DT, kind="ExternalInput"
    ).ap()
    lens = nc.dram_tensor("lens", [BPC, N_SENT], F32, kind="ExternalInput").ap()
    ident = nc.dram_tensor("ident", [128, 128], F32, kind="ExternalInput").ap()
    out = nc.dram_tensor(
        "out", [BPC, N_SENT, D_OUT], F32, kind="ExternalOutput"
    ).ap()

    with tile.TileContext(nc) as tc, ExitStack() as ctx:
        singles = ctx.enter_context(tc.tile_pool(name="singles", bufs=1))
        encp = ctx.enter_context(tc.tile_pool(name="encp", bufs=4))
        apool = ctx.enter_context(tc.tile_pool(name="apool", bufs=2))
        segp = ctx.enter_context(tc.tile_pool(name="segp", bufs=2))
        smalls = ctx.enter_context(tc.tile_pool(name="smalls", bufs=4))
        ps_seg = ctx.enter_context(tc.tile_pool(name="ps_seg", bufs=2, space="PSUM"))
        ps_tr = ctx.enter_context(tc.tile_pool(name="ps_tr", bufs=2, space="PSUM"))
        ps_pr = ctx.enter_context(tc.tile_pool(name="ps_pr", bufs=2, space="PSUM"))

        # ---- constants, loaded once ----
        w_sb = singles.tile([128, DCH, D_OUT], F32)
        nc.sync.dma_start(out=w_sb, in_=wt.rearrange("(j p) o -> p j o", p=128))
        ident_sb = singles.tile([128, 128], F32)
        nc.sync.dma_start(out=ident_sb, in_=ident)
        # b broadcast to [N_SENT, D_OUT] via stride-0 partition AP (SWDGE)
        b_bc = singles.tile([N_SENT, D_OUT], F32)
        bias_bcast = bass.AP(
            tensor=bias.tensor, offset=bias.offset,
            ap=[[0, N_SENT], [1, D_OUT]],
        )
        nc.gpsimd.dma_start(out=b_bc, in_=bias_bcast)
        # lens transposed into [N_SENT, BPC] so lens[:, bi] is a per-partition scalar
        lens_sb = singles.tile([N_SENT, BPC], F32)
        nc.sync.dma_start(out=lens_sb, in_=lens.rearrange("b n -> n b"))

        for bi in range(BPC):
            # segment-indicator matrix for this batch: [128(p), 32(k), 64(n)]
            a_sb = apool.tile([128, SCHUNKS, N_SENT], _SEG_MM_DT, tag="a")
            nc.sync.dma_start(
                out=a_sb, in_=amat[bi].rearrange("p (k n) -> p k n", n=N_SENT)
            )

            # ---- segment reduce: seg[n, d] = sum_s A[s, n] * enc[s, d] ----
            ps0 = ps_seg.tile([N_SENT, 512], F32, tag="ps0")
            ps1 = ps_seg.tile([N_SENT, 512], F32, tag="ps1")
            for kk in range(SCHUNKS // SS_PER_DMA):
                et = encp.tile([128, SS_PER_DMA, D_IN], _SEG_MM_DT, tag="enc")
                r0 = kk * 128 * SS_PER_DMA
                nc.sync.dma_start(
                    out=et,
                    in_=enc[bi, r0 : r0 + 128 * SS_PER_DMA, :].rearrange(
                        "(t p) d -> p t d", p=128
                    ),
                )
                for t in range(SS_PER_DMA):
                    k = kk * SS_PER_DMA + t
                    lhsT = a_sb[:, k, :]
                    for dh in range(2):
                        rhs = et[:, t, dh * 512 : (dh + 1) * 512]
                        nc.tensor.matmul(
                            ps0 if dh == 0 else ps1,
                            lhsT=lhsT,
                            rhs=rhs,
                            start=(k == 0),
                            stop=(k == SCHUNKS - 1),
                        )

            seg_sb = segp.tile([N_SENT, D_IN], F32, tag="seg")
            nc.vector.tensor_copy(out=seg_sb[:, 0:512], in_=ps0)
            nc.vector.tensor_copy(out=seg_sb[:, 512:1024], in_=ps1)

            # ---- transpose seg [64, 1024] -> segT [128(d), 8(j), 64(n)] ----
            seg_t = segp.tile([128, DCH, N_SENT], F32, tag="segT")
            for j in range(DCH):
                pt = ps_tr.tile([128, N_SENT], F32, tag="pt")
                nc.tensor.transpose(
                    out=pt,
                    in_=seg_sb[:, j * 128 : (j + 1) * 128],
                    identity=ident_sb[0:N_SENT, 0:N_SENT],
                )
                nc.vector.tensor_copy(out=seg_t[:, j, :], in_=pt)

            # ---- projection: sv[n, o] = sum_d segT[d, n] * W[d, o] ----
            pp = ps_pr.tile([N_SENT, D_OUT], F32, tag="pp")
            for j in range(DCH):
                nc.tensor.matmul(
                    pp,
                    lhsT=seg_t[:, j, :],
                    rhs=w_sb[:, j, :],
                    start=(j == 0),
                    stop=(j == DCH - 1),
                )

            # ---- sv += len * b ; log_softmax ----
            sv = smalls.tile([N_SENT, D_OUT], F32, tag="sv")
            nc.vector.scalar_tensor_tensor(
                out=sv,
                in0=b_bc,
                scalar=lens_sb[:, bi : bi + 1],
                in1=pp,
                op0=mybir.AluOpType.mult,
                op1=mybir.AluOpType.add,
            )
            negmax = smalls.tile([N_SENT, 1], F32, tag="negmax")
            nc.vector.tensor_reduce(
                out=negmax, in_=sv, axis=mybir.AxisListType.X,
                op=mybir.AluOpType.max, negate=True,
            )
            ex = smalls.tile([N_SENT, D_OUT], F32, tag="ex")
            ssum = smalls.tile([N_SENT, 1], F32, tag="ssum")
            nc.scalar.activation(
                out=ex, in_=sv, func=mybir.ActivationFunctionType.Exp,
                bias=negmax, scale=1.0, accum_out=ssum,
            )
            lse = smalls.tile([N_SENT, 1], F32, tag="lse")
            nc.scalar.activation(
                out=lse, in_=ssum, func=mybir.ActivationFunctionType.Ln
            )
            ot = smalls.tile([N_SENT, D_OUT], F32, tag="ot")
            nc.vector.tensor_scalar(
                out=ot, in0=sv, scalar1=negmax, scalar2=lse,
                op0=mybir.AluOpType.add, op1=mybir.AluOpType.subtract,
            )
            nc.sync.dma_start(out=out[bi], in_=ot)

    nc.compile()
    return nc


_PROGRAM = None


def _get_program():
    global _PROGRAM
    if _PROGRAM is None:
        _PROGRAM = _build_program()
    return _PROGRAM


def _host_prep(enc_output, W, b, cls_pos, last_sep):
    enc = np.ascontiguousarray(np.asarray(enc_output, dtype=np.float32))
    wf = np.ascontiguousarray(np.asarray(W, dtype=np.float32))
    bf = np.ascontiguousarray(np.asarray(b, dtype=np.float32))
    starts = np.asarray(cls_pos).astype(np.int64)                    # [B, N]
    lsep = np.asarray(last_sep).astype(np.int64)                     # [B]
    ends = np.concatenate([starts[:, 1:], (lsep + 1)[:, None]], axis=1)
    # torch semantics for the last segment: if end <= start, sum to seq end
    ends[:, -1] = np.where(ends[:, -1] > starts[:, -1], ends[:, -1], S)
    lens = (ends - starts).astype(np.float32)                        # [B, N]

    s = np.arange(S, dtype=np.int64)
    afull = (s[None, :, None] >= starts[:, None, :]) & (
        s[None, :, None] < ends[:, None, :]
    )                                                                # [B, S, N]
    amat = (
        afull.reshape(B, SCHUNKS, 128, N_SENT)
        .transpose(0, 2, 1, 3)
        .reshape(B, 128, SCHUNKS * N_SENT)
        .astype(np.float32)
    )
    return enc, wf, bf, amat, lens


def kernel(enc_output, W, b, max_num_sent, cls_pos, last_sep, _trace=False):
    enc, wf, bf, amat, lens = _host_prep(enc_output, W, b, cls_pos, last_sep)
    ident = np.eye(128, dtype=np.float32)

    nc = _get_program()
    in_maps = []
    for c in range(N_CORES):
        bsl = slice(c * BPC, (c + 1) * BPC)
        in_maps.append(
            {
                "enc": enc[bsl],
                "w": wf,
                "bias": bf,
                "amat": amat[bsl],
                "lens": lens[bsl],
                "ident": ident,
            }
        )
    res = bass_utils.run_bass_kernel_spmd(
        nc, in_maps, core_ids=list(range(N_CORES)), trace=_trace
    )
    out = np.concatenate(
        [res.results[c]["out"][None] for c in range(N_CORES)], axis=0
    ).reshape(B, N_SENT, D_OUT)
    if _trace:
        kernel._last_result = res
    return out.astype(np.float32)


# revision 9
# speedup vs baseline: 1.0781x; 1.0781x over previous
"""Trainium2 Bass kernel for nn_DialogActLabeller (segment_reduce).

Computes, for input enc_output [32, 4096, 1024], W [1024, 256], b [256],
cls_pos [32, 64], last_sep [32]:

    x = enc_output @ W + b                      # [B, S, 256]
    seg[b, n] = sum_{s in [start_n, end_n)} x[b, s, :]
    out = log_softmax(seg, axis=-1)             # [B, 64, 256]

Key algebraic restructure: the projection is linear, so segment-reduce
FIRST on enc_output (via a matmul with a 0/1 segment-indicator matrix A),
then project the tiny [64, 1024] per-batch result with W, and add
len_n * b for the bias.  This reads enc_output exactly once from HBM and
does ~1/32 of the naive FLOPs.

Sharding: pure data parallel, 4 batch rows per core across 8 cores
(W, b replicated), no cross-core communication.
"""

import os
import numpy as np

import concourse.bacc as bacc
import concourse.bass as bass
import concourse.tile as tile
from concourse import mybir
from concourse import bass_utils
from contextlib import ExitStack

# Problem shapes (hardcoded per contract)
B, S, D_IN, D_OUT, N_SENT = 32, 4096, 1024, 256, 64
N_CORES = 8
BPC = B // N_CORES          # batches per core
SCHUNKS = S // 128          # 32 sequence chunks of 128
DCH = D_IN // 128           # 8 d_in chunks of 128
SS_PER_DMA = 8              # s-chunks per enc DMA (4 MiB transfers)

F32 = mybir.dt.float32

# Matmul dtype for the big segment-reduce matmul and the small projection
# matmul.  float32r streams 4x faster through the PE than float32 on TRN2.
# Both operands are bitcast views of the same fp32 bits.
_SEG_MM_DT = getattr(mybir.dt, os.environ.get("SEG_MM_DT", "float32r"))
_PROJ_MM_DT = getattr(mybir.dt, os.environ.get("PROJ_MM_DT", "float32"))


def _build_program():
    nc = bacc.Bacc("TRN2", debug=False)

    # The segment-reduce matmul operands are declared end-to-end in the
    # matmul dtype (float32r is bit-identical to float32 in memory, so the
    # host still feeds plain fp32 arrays and the DMA is a plain copy).
    enc = nc.dram_tensor("enc", [BPC, S, D_IN], _SEG_MM_DT, kind="ExternalInput").ap()
    wt = nc.dram_tensor("w", [D_IN, D_OUT], F32, kind="ExternalInput").ap()
    bias = nc.dram_tensor("bias", [D_OUT], F32, kind="ExternalInput").ap()
    amat = nc.dram_tensor(
        "amat", [BPC, 128, SCHUNKS * N_SENT], _SEG_MM_DT, kind="ExternalInput"
    ).ap()
    lens = nc.dram_tensor("lens", [BPC, N_SENT], F32, kind="ExternalInput").ap()
    ident = nc.dram_tensor("ident", [128, 128], F32, kind="ExternalInput").ap()
    out = nc.dram_tensor(
        "out", [BPC, N_SENT, D_OUT], F32, kind="ExternalOutput"
    ).ap()

    with tile.TileContext(nc) as tc, ExitStack() as ctx:
        singles = ctx.enter_context(tc.tile_pool(name="singles", bufs=1))
        encp = ctx.enter_context(tc.tile_pool(name="encp", bufs=3))
        apool = ctx.enter_context(tc.tile_pool(name="apool", bufs=2))
        segp = ctx.enter_context(tc.tile_pool(name="segp", bufs=2))
        smalls = ctx.enter_context(tc.tile_pool(name="smalls", bufs=4))
        ps_seg = ctx.enter_context(tc.tile_pool(name="ps_seg", bufs=2, space="PSUM"))
        ps_tr = ctx.enter_context(tc.tile_pool(name="ps_tr", bufs=2, space="PSUM"))
        ps_pr = ctx.enter_context(tc.tile_pool(name="ps_pr", bufs=2, space="PSUM"))

        # ---- constants, loaded once ----
        w_sb = singles.tile([128, DCH, D_OUT], F32)
        nc.sync.dma_start(out=w_sb, in_=wt.rearrange("(j p) o -> p j o", p=128))
        ident_sb = singles.tile([128, 128], F32)
        nc.sync.dma_start(out=ident_sb, in_=ident)
        # b broadcast to [N_SENT, D_OUT] via stride-0 partition AP (SWDGE)
        b_bc = singles.tile([N_SENT, D_OUT], F32)
        bias_bcast = bass.AP(
            tensor=bias.tensor, offset=bias.offset,
            ap=[[0, N_SENT], [1, D_OUT]],
        )
        nc.gpsimd.dma_start(out=b_bc, in_=bias_bcast)
        # lens transposed into [N_SENT, BPC] so lens[:, bi] is a per-partition scalar
        lens_sb = singles.tile([N_SENT, BPC], F32)
        nc.sync.dma_start(out=lens_sb, in_=lens.rearrange("b n -> n b"))

        tails = []
        for bi in range(BPC):
            # segment-indicator matrix for this batch: [128(p), 32(k), 64(n)]
            a_sb = apool.tile([128, SCHUNKS, N_SENT], _SEG_MM_DT, tag="a")
            nc.sync.dma_start(
                out=a_sb, in_=amat[bi].rearrange("p (k n) -> p k n", n=N_SENT)
            )

            # ---- segment reduce: seg[n, d] = sum_s A[s, n] * enc[s, d] ----
            ps0 = ps_seg.tile([N_SENT, 512], F32, tag="ps0")
            ps1 = ps_seg.tile([N_SENT, 512], F32, tag="ps1")
            for kk in range(SCHUNKS // SS_PER_DMA):
                et = encp.tile([128, SS_PER_DMA, D_IN], _SEG_MM_DT, tag="enc")
                r0 = kk * 128 * SS_PER_DMA
                nc.sync.dma_start(
                    out=et,
                    in_=enc[bi, r0 : r0 + 128 * SS_PER_DMA, :].rearrange(
                        "(t p) d -> p t d", p=128
                    ),
                )
                for t in range(SS_PER_DMA):
                    k = kk * SS_PER_DMA + t
                    lhsT = a_sb[:, k, :]
                    for dh in range(2):
                        rhs = et[:, t, dh * 512 : (dh + 1) * 512]
                        nc.tensor.matmul(
                            ps0 if dh == 0 else ps1,
                            lhsT=lhsT,
                            rhs=rhs,
                            start=(k == 0),
                            stop=(k == SCHUNKS - 1),
                        )

            seg_sb = segp.tile([N_SENT, D_IN], F32, tag="seg")
            nc.vector.tensor_copy(out=seg_sb[:, 0:512], in_=ps0)
            nc.vector.tensor_copy(out=seg_sb[:, 512:1024], in_=ps1)

            # ---- transpose seg [64, 1024] -> segT [128(d), 8(j), 64(n)] ----
            seg_t = segp.tile([128, DCH, N_SENT], F32, tag="segT")
            for j in range(DCH):
                pt = ps_tr.tile([128, N_SENT], F32, tag="pt")
                nc.tensor.transpose(
                    out=pt,
                    in_=seg_sb[:, j * 128 : (j + 1) * 128],
                    identity=ident_sb[0:N_SENT, 0:N_SENT],
                )
                nc.vector.tensor_copy(out=seg_t[:, j, :], in_=pt)

            # ---- projection: sv[n, o] = sum_d segT[d, n] * W[d, o] ----
            pp = ps_pr.tile([N_SENT, D_OUT], F32, tag="pp")
            for j in range(DCH):
                nc.tensor.matmul(
                    pp,
                    lhsT=seg_t[:, j, :],
                    rhs=w_sb[:, j, :],
                    start=(j == 0),
                    stop=(j == DCH - 1),
                )

            # ---- sv = pp + len * b ; exp with running max/sum ----
            sv = smalls.tile([N_SENT, D_OUT], F32, tag=f"sv{bi}", bufs=1)
            nc.vector.scalar_tensor_tensor(
                out=sv,
                in0=b_bc,
                scalar=lens_sb[:, bi : bi + 1],
                in1=pp,
                op0=mybir.AluOpType.mult,
                op1=mybir.AluOpType.add,
            )
            negmax = smalls.tile([N_SENT, 1], F32, tag=f"negmax{bi}", bufs=1)
            nc.vector.tensor_reduce(
                out=negmax, in_=sv, axis=mybir.AxisListType.X,
                op=mybir.AluOpType.max, negate=True,
            )
            ex = smalls.tile([N_SENT, D_OUT], F32, tag="ex")
            ssum = smalls.tile([N_SENT, 1], F32, tag=f"ssum{bi}", bufs=1)
            nc.scalar.activation(
                out=ex, in_=sv, func=mybir.ActivationFunctionType.Exp,
                bias=negmax, scale=1.0, accum_out=ssum,
            )
            tails.append((sv, negmax, ssum))

        # ---- grouped log_softmax tail: one Ln table load for all batches ----
        for bi, (sv, negmax, ssum) in enumerate(tails):
            lse = smalls.tile([N_SENT, 1], F32, tag=f"lse{bi}", bufs=1)
            nc.scalar.activation(
                out=lse, in_=ssum, func=mybir.ActivationFunctionType.Ln
            )
            ot = smalls.tile([N_SENT, D_OUT], F32, tag="ot")
            nc.vector.tensor_scalar(
                out=ot, in0=sv, scalar1=negmax, scalar2=lse,
                op0=mybir.AluOpType.add, op1=mybir.AluOpType.subtract,
            )
            nc.sync.dma_start(out=out[bi], in_=ot)

    nc.compile()
    return nc


_PROGRAM = None


def _get_program():
    global _PROGRAM
    if _PROGRAM is None:
        _PROGRAM = _build_program()
    return _PROGRAM


def _host_prep(enc_output, W, b, cls_pos, last_sep):
    enc = np.ascontiguousarray(np.asarray(enc_output, dtype=np.float32))
    wf = np.ascontiguousarray(np.asarray(W, dtype=np.float32))
    bf = np.ascontiguousarray(np.asarray(b, dtype=np.float32))
    starts = np.asarray(cls_pos).astype(np.int64)                    # [B, N]
    lsep = np.asarray(last_sep).astype(np.int64)                     # [B]
    ends = np.concatenate([starts[:, 1:], (lsep + 1)[:, None]], axis=1)
    # torch semantics for the last segment: if end <= start, sum to seq end
    ends[:, -1] = np.where(ends[:, -1] > starts[:, -1], ends[:, -1], S)
    lens = (ends - starts).astype(np.float32)                        # [B, N]

    s = np.arange(S, dtype=np.int64)
    afull = (s[None, :, None] >= starts[:, None, :]) & (
        s[None, :, None] < ends[:, None, :]
    )                                                                # [B, S, N]
    amat = (
        afull.reshape(B, SCHUNKS, 128, N_SENT)
        .transpose(0, 2, 1, 3)
        .reshape(B, 128, SCHUNKS * N_SENT)
        .astype(np.float32)
    )
    return enc, wf, bf, amat, lens


def kernel(enc_output, W, b, max_num_sent, cls_pos, last_sep, _trace=False):
    enc, wf, bf, amat, lens = _host_prep(enc_output, W, b, cls_pos, last_sep)
    ident = np.eye(128, dtype=np.float32)

    nc = _get_program()
    in_maps = []
    for c in range(N_CORES):
        bsl = slice(c * BPC, (c + 1) * BPC)
        in_maps.append(
            {
                "enc": enc[bsl],
                "w": wf,
                "bias": bf,
                "amat": amat[bsl],
                "lens": lens[bsl],
                "ident": ident,
            }
        )
    res = bass_utils.run_bass_kernel_spmd(
        nc, in_maps, core_ids=list(range(N_CORES)), trace=_trace
    )
    out = np.concatenate(
        [res.results[c]["out"][None] for c in range(N_CORES)], axis=0
    ).reshape(B, N_SENT, D_OUT)
    if _trace:
        kernel._last_result = res
    return out.astype(np.float32)
